# revision 18
# baseline (speedup 1.0000x reference)
"""Trainium2 Bass kernel for batched filtfilt band-pass filtering (tensorpac-style).

Math: scipy-style filtfilt with FIR taps b is (exactly) a single convolution of
the odd-extended input with the autocorrelation of b, evaluated on the interior:

    out[n] = sum_d A[d] * ext[P + n + d],   d in [-(t-1), t-1]
    A[d]   = sum_i b[i] * b[i+d]            (t = effective tap count)

provided padlen P >= t-1 (true here: P = 512, t <= 513). The left "lfilter_zi"
constant extension and the right-edge extension of the backward pass never reach
the retained [P, P+L) window, so the equivalence is exact (verified to 1e-16).

Device mapping (per core, sequence-parallel over 8 cores):
  - each core owns 2048 output positions x all 128 batches; its input is a
    (3072, 128) slice of ext^T (position-major) covering the 2x512 halo,
    shipped fp16 in the SBUF-native [partition, h-block, batch] layout.
  - out[r, (j,b)] tiles (128 positions x 4 pos-blocks x 128 batches) accumulate
    in fp32 PSUM via K=128 fp16 matmuls: lhsT = 128x128 banded-Toeplitz blocks
    of A (host-precomputed fp16 constants), rhs = 512-wide slices of ext^T.
  - per band, the number of Toeplitz blocks adapts to the true tap support
    (Q = ceil((2t+126)/128)); a half-block-shifted copy of ext^T (E64, built
    on-device from E via two partition-shifted SBUF->SBUF DMAs) lets short
    bands cover their diagonal band with Q = minimal block count.
  - loop is band-outer so the per-band constant stream (2.1 MB) overlaps the
    matmul phase; outputs stream out per (band, 512-position group) as 256 KB
    contiguous DMAs on the ACT HWDGE ring while inputs use the SP ring.
"""

import os

import numpy as np

import concourse.bass as bass
import concourse.mybir as mybir
from concourse import bacc
from concourse.tile import TileContext
from concourse.bass_utils import run_bass_kernel_spmd

F32 = mybir.dt.float32
F16 = mybir.dt.float16

B = 128          # batch
L = 16384        # sequence length
P = 512          # padlen (= TAPS - 1)
NB = 20          # bands
N_CORES = 8
LC = L // N_CORES            # 2048 output positions per core
GROUPS = LC // 512           # 4 groups of 512 positions
EXT_ROWS = LC + 2 * P        # 3072 ext rows per core (halo included)
H_E = EXT_ROWS // 128        # 24 aligned 128-row blocks
H_E64 = (EXT_ROWS - 128) // 128  # 23 half-shifted blocks (rows 64 + 128h + p)
N_WARM = 20                  # dummy matmuls to warm the PE HAM during input DMA

LAST_RESULT = None  # BassKernelResults of the most recent run (for test harness)

_program_cache: dict = {}


def _band_plan(kernels: np.ndarray):
    """Per-band tap support -> (t, Q, s, use64, h_base) block plan.

    Block q covers ext rows m = n0 + P - s + 128q + kk (kk = partition), so
    diagonal d = 128q + kk - s - r. Coverage of d in [-(t-1), t-1] for every
    r in [0,128) requires s >= t-1 and s <= 128Q - 127 - t. s is the smallest
    multiple of 64 >= t-1; s % 128 == 64 uses the half-shifted E64 copy.
    """
    plan = []
    for k in range(kernels.shape[0]):
        nz = np.nonzero(kernels[k])[0]
        t = int(nz[-1]) + 1 if nz.size else 1
        assert t - 1 <= P, f"band {k}: taps {t} exceed padlen {P}"
        q_cnt = (2 * t + 126 + 127) // 128
        s = 64 * ((t - 1 + 63) // 64) if t > 1 else 0
        assert s >= t - 1 and s <= 128 * q_cnt - 127 - t, (k, t, q_cnt, s)
        use64 = (s % 128) == 64
        if use64:
            h_base = (P - 64 - s) // 128
        else:
            h_base = (P - s) // 128
        assert h_base >= 0
        plan.append((t, q_cnt, s, use64, h_base))
    return plan


def _band_order(plan):
    """First a small aligned band (so the out-DMA stream starts as early as
    possible), then interleave large- and small-Q bands so the PSUM->SBUF
    copy and out-DMA load stays smooth. E64 bands are kept out of the first
    two slots to cover the on-device E64 build latency."""
    by_q = sorted(range(len(plan)), key=lambda k: (-plan[k][1], plan[k][3]))
    small_aligned = [k for k in by_q if not plan[k][3]]
    first = small_aligned[-1] if small_aligned else by_q[0]
    rest = [k for k in by_q if k != first]
    order, lo, hi = [first], 0, len(rest) - 1
    while lo <= hi:
        order.append(rest[lo]); lo += 1
        if lo <= hi:
            order.append(rest[hi]); hi -= 1
    return order


def _toeplitz_blocks(kernels: np.ndarray, plan):
    """Stacked lhsT blocks in SBUF-native layout: (128, NBLK, 128) fp16,
    [kk, block, r] with the contraction dim kk on axis 0."""
    nblk = sum(p[1] for p in plan)
    out = np.zeros((128, nblk, 128), np.float16)
    kk = np.arange(128)[:, None]
    rr = np.arange(128)[None, :]
    i = 0
    for k, (t, q_cnt, s, _use64, _hb) in enumerate(plan):
        bk = kernels[k][:t].astype(np.float64)
        acorr = np.correlate(bk, bk, mode="full")  # length 2t-1, center t-1
        a_full = np.zeros(2 * P + 1, np.float64)
        a_full[P - (t - 1) : P + t] = acorr
        for q in range(q_cnt):
            d = 128 * q - s + kk - rr
            valid = (d >= -(t - 1)) & (d <= t - 1)
            blk = np.where(valid, a_full[np.clip(d + P, 0, 2 * P)], 0.0)
            out[:, i, :] = blk.astype(np.float16)
            i += 1
    return out


def _build_program(plan_key):
    """Compile the SPMD program for a given block structure. Cached."""
    if plan_key in _program_cache:
        return _program_cache[plan_key]

    plan = list(plan_key)
    offsets = np.cumsum([0] + [p[1] for p in plan]).tolist()
    nblk = offsets[-1]
    order = _band_order(plan)

    nc = bacc.Bacc("TRN2", target_bir_lowering=False, debug=False,
                   num_devices=N_CORES)
    # host-permuted ext^T slice: [p, h, b] fp16 (SBUF-native layout)
    ext_in = nc.declare_dram_parameter("ext", [128, H_E, B], F16, isOutput=False)
    e64_in = nc.declare_dram_parameter("e64", [128, H_E64, B], F16,
                                       isOutput=False)
    lhs_in = nc.declare_dram_parameter("lhs", [128, nblk, 128], F16,
                                       isOutput=False)
    out_t = nc.declare_dram_parameter("out", [NB, 128, GROUPS * 512], F32,
                                      isOutput=True)

    with TileContext(nc) as tc:
        with (
            tc.tile_pool(name="consts", bufs=1) as cpool,
            tc.tile_pool(name="psum", bufs=8, space="PSUM") as ppool,
            tc.tile_pool(name="ostage", bufs=6) as opool,
        ):
            E = cpool.tile([128, H_E * 128], F16)
            E64 = cpool.tile([128, H_E64 * 128], F16)
            Lw = cpool.tile([128, nblk * 128], F16)
            warm = cpool.tile([128, 256], F16)
            wps = ppool.tile([128, 512], F32, tag="ps")

            # PE warm-up during the input DMAs: harmless matmuls on a zeroed
            # tile keep the HAM busy window alive so real matmuls start warm.
            nc.any.memset(warm[:], 0.0)
            for w in range(N_WARM):
                nc.tensor.matmul(wps[:, 0:256], warm[:, :128], warm[:],
                                 start=True, stop=True)

            # E in 2 chunks -> 2 DMA queues, so the critical first input
            # does not fair-share bandwidth with the constant stream
            e_flat = ext_in[:].rearrange("p h b -> p (h b)")
            chunk = (H_E // 2) * 128
            for ci in range(2):
                nc.sync.dma_start(out=E[:, ci * chunk : (ci + 1) * chunk],
                                  in_=e_flat[:, ci * chunk : (ci + 1) * chunk])
            # E64 (half-block-shifted ext^T copy) ships from the host: at
            # fp16 it is only 0.72 MB of HBM, cheaper than the 2.8 MB of
            # SBUF<->SBUF fabric work an on-device build would cost.
            nc.sync.dma_start(out=E64[:],
                              in_=e64_in[:].rearrange("p h b -> p (h b)"))

            # all constants stream on the Sync ring in band order, QUEUED
            # BEHIND the E64 triggers whose sem-wait on E head-of-line blocks
            # the ring: E gets exclusive DMA bandwidth, then constants stream
            # at the ring's natural ~0.6us/trigger pace.
            for idx, k in enumerate(order):
                o, q_cnt = offsets[k], plan[k][1]
                nc.sync.dma_start(
                    out=Lw[:, o * 128 : (o + q_cnt) * 128].rearrange(
                        "kk (i r) -> kk i r", r=128
                    ),
                    in_=lhs_in[:, o : o + q_cnt, :],
                )

            for k in order:
                t, q_cnt, s, use64, h_base = plan[k]
                o = offsets[k]
                src = E64 if use64 else E
                h_max = H_E64 if use64 else H_E
                ob = opool.tile([128, GROUPS * 512], F32)
                for g in range(GROUPS):
                    ps = ppool.tile([128, 512], F32)
                    for qi in range(q_cnt):
                        h0 = 4 * g + h_base + qi
                        assert 0 <= h0 and h0 + 4 <= h_max, (k, g, qi, h0)
                        nc.tensor.matmul(
                            ps[:],
                            Lw[:, (o + qi) * 128 : (o + qi + 1) * 128],
                            src[:, h0 * 128 : h0 * 128 + 512],
                            start=(qi == 0),
                            stop=(qi == q_cnt - 1),
                        )
                    # split the PSUM drain across DVE and ACT so neither
                    # engine gates the PSUM bank turnaround
                    base = g * 512
                    nc.vector.tensor_copy(ob[:, base : base + 384], ps[:, 0:384])
                    nc.scalar.copy(ob[:, base + 384 : base + 512], ps[:, 384:512])
                    if k == order[-1]:
                        nc.sync.dma_start(
                            out=out_t[k, :, base : base + 512],
                            in_=ob[:, base : base + 512],
                        )
                # one contiguous 1 MB out-DMA per band (trigger cost ~0.6us
                # on the issuing engine, so fewer+bigger is better); the final
                # band streams per-group so the kernel tail is short
                if k == order[-1]:
                    pass  # handled per-group above
                else:
                    nc.sync.dma_start(out=out_t[k], in_=ob[:])

    nc.compile()
    _program_cache[plan_key] = nc
    return nc


def _maybe_register_trace_hook():
    """Best-effort registration of the axon NTFF profile hook (profiling only;
    harmless no-op if unavailable)."""
    try:
        import sys
        import types

        import antenv

        if getattr(antenv, "axon_hooks", None) is not None:
            return
        from trn_agent_boot.trn_boot import _ntff_profile_via_ctypes

        hooks = types.ModuleType("antenv.axon_hooks")
        hook = _ntff_profile_via_ctypes("/opt/axon/libaxon_pjrt.so")
        hooks.get_axon_ntff_profile_hook = lambda: hook
        hooks.set_axon_ntff_profile_hook = lambda h: None
        antenv.axon_hooks = hooks
        sys.modules["antenv.axon_hooks"] = hooks
    except Exception:
        pass


def kernel(x: np.ndarray, kernels: np.ndarray, padlen) -> np.ndarray:
    global LAST_RESULT
    x = np.asarray(x, dtype=np.float32)
    kernels = np.asarray(kernels, dtype=np.float32)
    assert x.shape == (B, 1, L) and kernels.shape[0] == NB
    assert int(padlen) == P

    plan = _band_plan(kernels)
    plan_key = tuple(plan)
    nc = _build_program(plan_key)

    lhs = np.ascontiguousarray(_toeplitz_blocks(kernels, plan))

    # odd extension + transpose to position-major (ext^T), fp16
    x2d = x[:, 0, :]
    left = 2.0 * x2d[:, :1] - x2d[:, 1 : P + 1][:, ::-1]
    right = 2.0 * x2d[:, -1:] - x2d[:, -P - 1 : -1][:, ::-1]
    ext_t = np.concatenate([left, x2d, right], axis=1).T.astype(np.float16)

    in_maps = []
    for c in range(N_CORES):
        sl = ext_t[c * LC : c * LC + EXT_ROWS]  # (3072, B)
        # SBUF-native layout [p, h, b]: row (128h + p) -> [p, h]
        slp = np.ascontiguousarray(
            sl.reshape(H_E, 128, B).transpose(1, 0, 2)
        )
        sl64 = sl[64 : 64 + H_E64 * 128]
        slp64 = np.ascontiguousarray(
            sl64.reshape(H_E64, 128, B).transpose(1, 0, 2)
        )
        in_maps.append({"ext": slp, "e64": slp64, "lhs": lhs})

    trace = bool(os.environ.get("KERNEL_TRACE"))
    if trace:
        _maybe_register_trace_hook()
    res = run_bass_kernel_spmd(nc, in_maps, list(range(N_CORES)), trace=trace)
    LAST_RESULT = res

    out = np.empty((B, 1, NB, L), np.float32)
    for c in range(N_CORES):
        dev = res.results[c]["out"].reshape(NB, 128, GROUPS, 4, 128)
        # dev[k, r, g, j, b] -> out[b, 0, k, c*LC + 512g + 128j + r]
        arr = dev.transpose(4, 0, 2, 3, 1).reshape(B, NB, LC)
        out[:, 0, :, c * LC : (c + 1) * LC] = arr
    return out


# revision 19
# speedup vs baseline: 1.2296x; 1.2296x over previous
"""Trainium2 Bass kernel for batched filtfilt band-pass filtering (tensorpac-style).

Math: scipy-style filtfilt with FIR taps b is (exactly) a single convolution of
the odd-extended input with the autocorrelation of b, evaluated on the interior:

    out[n] = sum_d A[d] * ext[P + n + d],   d in [-(t-1), t-1]
    A[d]   = sum_i b[i] * b[i+d]            (t = effective tap count)

provided padlen P >= t-1 (true here: P = 512, t <= 513). The left "lfilter_zi"
constant extension and the right-edge extension of the backward pass never reach
the retained [P, P+L) window, so the equivalence is exact (verified to 1e-16).

Device mapping (per core, sequence-parallel over 8 cores):
  - each core owns 2048 output positions x all 128 batches; its input is a
    (3072, 128) slice of ext^T (position-major) covering the 2x512 halo,
    shipped fp16 in the SBUF-native [partition, h-block, batch] layout.
  - out[r, (j,b)] tiles (128 positions x 4 pos-blocks x 128 batches) accumulate
    in fp32 PSUM via K=128 fp16 matmuls: lhsT = 128x128 banded-Toeplitz blocks
    of A (host-precomputed fp16 constants), rhs = 512-wide slices of ext^T.
  - per band, the number of Toeplitz blocks adapts to the true tap support
    (Q = ceil((2t+126)/128)); a half-block-shifted copy of ext^T (E64, built
    on-device from E via two partition-shifted SBUF->SBUF DMAs) lets short
    bands cover their diagonal band with Q = minimal block count.
  - loop is band-outer so the per-band constant stream (2.1 MB) overlaps the
    matmul phase; outputs stream out per (band, 512-position group) as 256 KB
    contiguous DMAs on the ACT HWDGE ring while inputs use the SP ring.
"""

import os

import numpy as np

import concourse.bass as bass
import concourse.mybir as mybir
from concourse import bacc
from concourse.tile import TileContext
from concourse.bass_utils import run_bass_kernel_spmd

F32 = mybir.dt.float32
F16 = mybir.dt.float16

B = 128          # batch
L = 16384        # sequence length
P = 512          # padlen (= TAPS - 1)
NB = 20          # bands
N_CORES = 8
LC = L // N_CORES            # 2048 output positions per core
GROUPS = LC // 512           # 4 groups of 512 positions
EXT_ROWS = LC + 2 * P        # 3072 ext rows per core (halo included)
H_E = EXT_ROWS // 128        # 24 aligned 128-row blocks
H_E64 = (EXT_ROWS - 128) // 128  # 23 half-shifted blocks (rows 64 + 128h + p)
N_WARM = 20                  # dummy matmuls to warm the PE HAM during input DMA

LAST_RESULT = None  # BassKernelResults of the most recent run (for test harness)

_program_cache: dict = {}


def _band_plan(kernels: np.ndarray):
    """Per-band tap support -> (t, Q, s, use64, h_base) block plan.

    Block q covers ext rows m = n0 + P - s + 128q + kk (kk = partition), so
    diagonal d = 128q + kk - s - r. Coverage of d in [-(t-1), t-1] for every
    r in [0,128) requires s >= t-1 and s <= 128Q - 127 - t. s is the smallest
    multiple of 64 >= t-1; s % 128 == 64 uses the half-shifted E64 copy.
    """
    plan = []
    for k in range(kernels.shape[0]):
        nz = np.nonzero(kernels[k])[0]
        t = int(nz[-1]) + 1 if nz.size else 1
        assert t - 1 <= P, f"band {k}: taps {t} exceed padlen {P}"
        q_cnt = (2 * t + 126 + 127) // 128
        s = 64 * ((t - 1 + 63) // 64) if t > 1 else 0
        assert s >= t - 1 and s <= 128 * q_cnt - 127 - t, (k, t, q_cnt, s)
        use64 = (s % 128) == 64
        if use64:
            h_base = (P - 64 - s) // 128
        else:
            h_base = (P - s) // 128
        assert h_base >= 0
        plan.append((t, q_cnt, s, use64, h_base))
    return plan


def _band_order(plan):
    """First a small aligned band (so the out-DMA stream starts as early as
    possible), then interleave large- and small-Q bands so the PSUM->SBUF
    copy and out-DMA load stays smooth. E64 bands are kept out of the first
    two slots to cover the on-device E64 build latency."""
    by_q = sorted(range(len(plan)), key=lambda k: (-plan[k][1], plan[k][3]))
    small_aligned = [k for k in by_q if not plan[k][3]]
    first = small_aligned[-1] if small_aligned else by_q[0]
    rest = [k for k in by_q if k != first]
    order, lo, hi = [first], 0, len(rest) - 1
    while lo <= hi:
        order.append(rest[lo]); lo += 1
        if lo <= hi:
            order.append(rest[hi]); hi -= 1
    return order


def _toeplitz_blocks(kernels: np.ndarray, plan):
    """Stacked lhsT blocks in SBUF-native layout: (128, NBLK, 128) fp16,
    [kk, block, r] with the contraction dim kk on axis 0."""
    nblk = sum(p[1] for p in plan)
    out = np.zeros((128, nblk, 128), np.float16)
    kk = np.arange(128)[:, None]
    rr = np.arange(128)[None, :]
    i = 0
    for k, (t, q_cnt, s, _use64, _hb) in enumerate(plan):
        bk = kernels[k][:t].astype(np.float64)
        acorr = np.correlate(bk, bk, mode="full")  # length 2t-1, center t-1
        a_full = np.zeros(2 * P + 1, np.float64)
        a_full[P - (t - 1) : P + t] = acorr
        for q in range(q_cnt):
            d = 128 * q - s + kk - rr
            valid = (d >= -(t - 1)) & (d <= t - 1)
            blk = np.where(valid, a_full[np.clip(d + P, 0, 2 * P)], 0.0)
            out[:, i, :] = blk.astype(np.float16)
            i += 1
    return out


def _build_program(plan_key):
    """Compile the SPMD program for a given block structure. Cached."""
    if plan_key in _program_cache:
        return _program_cache[plan_key]

    plan = list(plan_key)
    offsets = np.cumsum([0] + [p[1] for p in plan]).tolist()
    nblk = offsets[-1]
    order = _band_order(plan)

    nc = bacc.Bacc("TRN2", target_bir_lowering=False, debug=False,
                   num_devices=N_CORES)
    # host-permuted ext^T slice: [p, h, b] fp16 (SBUF-native layout)
    ext_in = nc.declare_dram_parameter("ext", [128, H_E, B], F16, isOutput=False)
    lhs_in = nc.declare_dram_parameter("lhs", [128, nblk, 128], F16,
                                       isOutput=False)
    out_t = nc.declare_dram_parameter("out", [NB, 128, GROUPS * 512], F16,
                                      isOutput=True)

    with TileContext(nc) as tc:
        with (
            tc.tile_pool(name="consts", bufs=1) as cpool,
            tc.tile_pool(name="psum", bufs=8, space="PSUM") as ppool,
            tc.tile_pool(name="ostage", bufs=6) as opool,
        ):
            E = cpool.tile([128, H_E * 128], F16)
            E64 = cpool.tile([128, H_E64 * 128], F16)
            Lw = cpool.tile([128, nblk * 128], F16)
            warm = cpool.tile([128, 256], F16)
            wps = ppool.tile([128, 512], F32, tag="ps")

            # PE warm-up during the input DMAs: harmless matmuls on a zeroed
            # tile keep the HAM busy window alive so real matmuls start warm.
            nc.any.memset(warm[:], 0.0)
            for w in range(N_WARM):
                nc.tensor.matmul(wps[:, 0:256], warm[:, :128], warm[:],
                                 start=True, stop=True)

            # E in 2 chunks -> 2 DMA queues, so the critical first input
            # does not fair-share bandwidth with the constant stream
            e_flat = ext_in[:].rearrange("p h b -> p (h b)")
            chunk = (H_E // 2) * 128
            for ci in range(2):
                nc.sync.dma_start(out=E[:, ci * chunk : (ci + 1) * chunk],
                                  in_=e_flat[:, ci * chunk : (ci + 1) * chunk])
            # E64[p, h] = ext rows (64 + 128h + p), built on device from E.
            # The sem-wait of these triggers head-of-line blocks the HWDGE
            # ring, which (deliberately) gives E exclusive DMA bandwidth.
            e3 = E[:].rearrange("p (h b) -> p h b", b=B)
            e643 = E64[:].rearrange("p (h b) -> p h b", b=B)
            nc.sync.dma_start(out=e643[0:64, :, :], in_=e3[64:128, 0:H_E64, :])
            nc.sync.dma_start(out=e643[64:128, :, :], in_=e3[0:64, 1 : H_E64 + 1, :])

            # all constants stream on the Sync ring in band order, QUEUED
            # BEHIND the E64 triggers whose sem-wait on E head-of-line blocks
            # the ring: E gets exclusive DMA bandwidth, then constants stream
            # at the ring's natural ~0.6us/trigger pace.
            for idx, k in enumerate(order):
                o, q_cnt = offsets[k], plan[k][1]
                nc.sync.dma_start(
                    out=Lw[:, o * 128 : (o + q_cnt) * 128].rearrange(
                        "kk (i r) -> kk i r", r=128
                    ),
                    in_=lhs_in[:, o : o + q_cnt, :],
                )

            for k in order:
                t, q_cnt, s, use64, h_base = plan[k]
                o = offsets[k]
                src = E64 if use64 else E
                h_max = H_E64 if use64 else H_E
                ob = opool.tile([128, GROUPS * 512], F16)
                for g in range(GROUPS):
                    ps = ppool.tile([128, 512], F32)
                    for qi in range(q_cnt):
                        h0 = 4 * g + h_base + qi
                        assert 0 <= h0 and h0 + 4 <= h_max, (k, g, qi, h0)
                        nc.tensor.matmul(
                            ps[:],
                            Lw[:, (o + qi) * 128 : (o + qi + 1) * 128],
                            src[:, h0 * 128 : h0 * 128 + 512],
                            start=(qi == 0),
                            stop=(qi == q_cnt - 1),
                        )
                    # split the PSUM drain across DVE and ACT so neither
                    # engine gates the PSUM bank turnaround
                    base = g * 512
                    nc.vector.tensor_copy(ob[:, base : base + 384], ps[:, 0:384])
                    nc.scalar.copy(ob[:, base + 384 : base + 512], ps[:, 384:512])
                    if k == order[-1]:
                        nc.sync.dma_start(
                            out=out_t[k, :, base : base + 512],
                            in_=ob[:, base : base + 512],
                        )
                # one contiguous 1 MB out-DMA per band (trigger cost ~0.6us
                # on the issuing engine, so fewer+bigger is better); the final
                # band streams per-group so the kernel tail is short
                if k == order[-1]:
                    pass  # handled per-group above
                else:
                    nc.sync.dma_start(out=out_t[k], in_=ob[:])

    nc.compile()
    _program_cache[plan_key] = nc
    return nc


def _maybe_register_trace_hook():
    """Best-effort registration of the axon NTFF profile hook (profiling only;
    harmless no-op if unavailable)."""
    try:
        import sys
        import types

        import antenv

        if getattr(antenv, "axon_hooks", None) is not None:
            return
        from trn_agent_boot.trn_boot import _ntff_profile_via_ctypes

        hooks = types.ModuleType("antenv.axon_hooks")
        hook = _ntff_profile_via_ctypes("/opt/axon/libaxon_pjrt.so")
        hooks.get_axon_ntff_profile_hook = lambda: hook
        hooks.set_axon_ntff_profile_hook = lambda h: None
        antenv.axon_hooks = hooks
        sys.modules["antenv.axon_hooks"] = hooks
    except Exception:
        pass


def kernel(x: np.ndarray, kernels: np.ndarray, padlen) -> np.ndarray:
    global LAST_RESULT
    x = np.asarray(x, dtype=np.float32)
    kernels = np.asarray(kernels, dtype=np.float32)
    assert x.shape == (B, 1, L) and kernels.shape[0] == NB
    assert int(padlen) == P

    plan = _band_plan(kernels)
    plan_key = tuple(plan)
    nc = _build_program(plan_key)

    lhs = np.ascontiguousarray(_toeplitz_blocks(kernels, plan))

    # odd extension + transpose to position-major (ext^T), fp16
    x2d = x[:, 0, :]
    left = 2.0 * x2d[:, :1] - x2d[:, 1 : P + 1][:, ::-1]
    right = 2.0 * x2d[:, -1:] - x2d[:, -P - 1 : -1][:, ::-1]
    ext_t = np.concatenate([left, x2d, right], axis=1).T.astype(np.float16)

    in_maps = []
    for c in range(N_CORES):
        sl = ext_t[c * LC : c * LC + EXT_ROWS]  # (3072, B)
        # SBUF-native layout [p, h, b]: row (128h + p) -> [p, h]
        slp = np.ascontiguousarray(
            sl.reshape(H_E, 128, B).transpose(1, 0, 2)
        )
        in_maps.append({"ext": slp, "lhs": lhs})

    trace = bool(os.environ.get("KERNEL_TRACE"))
    if trace:
        _maybe_register_trace_hook()
    res = run_bass_kernel_spmd(nc, in_maps, list(range(N_CORES)), trace=trace)
    LAST_RESULT = res

    out = np.empty((B, 1, NB, L), np.float32)
    for c in range(N_CORES):
        dev = res.results[c]["out"].astype(np.float32).reshape(NB, 128, GROUPS, 4, 128)
        # dev[k, r, g, j, b] -> out[b, 0, k, c*LC + 512g + 128j + r]
        arr = dev.transpose(4, 0, 2, 3, 1).reshape(B, NB, LC)
        out[:, 0, :, c * LC : (c + 1) * LC] = arr
    return out


# revision 20
# speedup vs baseline: 1.2362x; 1.0054x over previous
"""Trainium2 Bass kernel for batched filtfilt band-pass filtering (tensorpac-style).

Math: scipy-style filtfilt with FIR taps b is (exactly) a single convolution of
the odd-extended input with the autocorrelation of b, evaluated on the interior:

    out[n] = sum_d A[d] * ext[P + n + d],   d in [-(t-1), t-1]
    A[d]   = sum_i b[i] * b[i+d]            (t = effective tap count)

provided padlen P >= t-1 (true here: P = 512, t <= 513). The left "lfilter_zi"
constant extension and the right-edge extension of the backward pass never reach
the retained [P, P+L) window, so the equivalence is exact (verified to 1e-16).

Device mapping (per core, sequence-parallel over 8 cores):
  - each core owns 2048 output positions x all 128 batches; its input is a
    (3072, 128) slice of ext^T (position-major) covering the 2x512 halo,
    shipped fp16 in the SBUF-native [partition, h-block, batch] layout.
  - out[r, (j,b)] tiles (128 positions x 4 pos-blocks x 128 batches) accumulate
    in fp32 PSUM via K=128 fp16 matmuls: lhsT = 128x128 banded-Toeplitz blocks
    of A (host-precomputed fp16 constants), rhs = 512-wide slices of ext^T.
  - per band, the number of Toeplitz blocks adapts to the true tap support
    (Q = ceil((2t+126)/128)); a half-block-shifted copy of ext^T (E64, built
    on-device from E via two partition-shifted SBUF->SBUF DMAs) lets short
    bands cover their diagonal band with Q = minimal block count.
  - loop is band-outer so the per-band constant stream (2.1 MB) overlaps the
    matmul phase; outputs stream out per (band, 512-position group) as 256 KB
    contiguous DMAs on the ACT HWDGE ring while inputs use the SP ring.
"""

import os

import numpy as np

import concourse.bass as bass
import concourse.mybir as mybir
from concourse import bacc
from concourse.tile import TileContext
from concourse.bass_utils import run_bass_kernel_spmd

F32 = mybir.dt.float32
F16 = mybir.dt.float16

B = 128          # batch
L = 16384        # sequence length
P = 512          # padlen (= TAPS - 1)
NB = 20          # bands
N_CORES = 8
LC = L // N_CORES            # 2048 output positions per core
GROUPS = LC // 512           # 4 groups of 512 positions
EXT_ROWS = LC + 2 * P        # 3072 ext rows per core (halo included)
H_E = EXT_ROWS // 128        # 24 aligned 128-row blocks
H_E64 = (EXT_ROWS - 128) // 128  # 23 half-shifted blocks (rows 64 + 128h + p)
N_WARM = 10                  # dummy matmuls to warm the PE HAM during input DMA

LAST_RESULT = None  # BassKernelResults of the most recent run (for test harness)

_program_cache: dict = {}


def _band_plan(kernels: np.ndarray):
    """Per-band tap support -> (t, Q, s, use64, h_base) block plan.

    Block q covers ext rows m = n0 + P - s + 128q + kk (kk = partition), so
    diagonal d = 128q + kk - s - r. Coverage of d in [-(t-1), t-1] for every
    r in [0,128) requires s >= t-1 and s <= 128Q - 127 - t. s is the smallest
    multiple of 64 >= t-1; s % 128 == 64 uses the half-shifted E64 copy.
    """
    plan = []
    for k in range(kernels.shape[0]):
        nz = np.nonzero(kernels[k])[0]
        t = int(nz[-1]) + 1 if nz.size else 1
        assert t - 1 <= P, f"band {k}: taps {t} exceed padlen {P}"
        q_cnt = (2 * t + 126 + 127) // 128
        s = 64 * ((t - 1 + 63) // 64) if t > 1 else 0
        assert s >= t - 1 and s <= 128 * q_cnt - 127 - t, (k, t, q_cnt, s)
        use64 = (s % 128) == 64
        if use64:
            h_base = (P - 64 - s) // 128
        else:
            h_base = (P - s) // 128
        assert h_base >= 0
        plan.append((t, q_cnt, s, use64, h_base))
    return plan


def _band_order(plan):
    """First a small aligned band (so the out-DMA stream starts as early as
    possible), then interleave large- and small-Q bands so the PSUM->SBUF
    copy and out-DMA load stays smooth. E64 bands are kept out of the first
    two slots to cover the on-device E64 build latency."""
    by_q = sorted(range(len(plan)), key=lambda k: (-plan[k][1], plan[k][3]))
    small_aligned = [k for k in by_q if not plan[k][3]]
    first = small_aligned[-1] if small_aligned else by_q[0]
    rest = [k for k in by_q if k != first]
    order, lo, hi = [first], 0, len(rest) - 1
    while lo <= hi:
        order.append(rest[lo]); lo += 1
        if lo <= hi:
            order.append(rest[hi]); hi -= 1
    return order


def _toeplitz_blocks(kernels: np.ndarray, plan):
    """Stacked lhsT blocks in SBUF-native layout: (128, NBLK, 128) fp16,
    [kk, block, r] with the contraction dim kk on axis 0."""
    nblk = sum(p[1] for p in plan)
    out = np.zeros((128, nblk, 128), np.float16)
    kk = np.arange(128)[:, None]
    rr = np.arange(128)[None, :]
    i = 0
    for k, (t, q_cnt, s, _use64, _hb) in enumerate(plan):
        bk = kernels[k][:t].astype(np.float64)
        acorr = np.correlate(bk, bk, mode="full")  # length 2t-1, center t-1
        a_full = np.zeros(2 * P + 1, np.float64)
        a_full[P - (t - 1) : P + t] = acorr
        for q in range(q_cnt):
            d = 128 * q - s + kk - rr
            valid = (d >= -(t - 1)) & (d <= t - 1)
            blk = np.where(valid, a_full[np.clip(d + P, 0, 2 * P)], 0.0)
            out[:, i, :] = blk.astype(np.float16)
            i += 1
    return out


def _build_program(plan_key):
    """Compile the SPMD program for a given block structure. Cached."""
    if plan_key in _program_cache:
        return _program_cache[plan_key]

    plan = list(plan_key)
    offsets = np.cumsum([0] + [p[1] for p in plan]).tolist()
    nblk = offsets[-1]
    order = _band_order(plan)

    nc = bacc.Bacc("TRN2", target_bir_lowering=False, debug=False,
                   num_devices=N_CORES)
    # host-permuted ext^T slice: [p, h, b] fp16 (SBUF-native layout)
    ext_in = nc.declare_dram_parameter("ext", [128, H_E, B], F16, isOutput=False)
    lhs_in = nc.declare_dram_parameter("lhs", [128, nblk, 128], F16,
                                       isOutput=False)
    out_t = nc.declare_dram_parameter("out", [NB, 128, GROUPS * 512], F16,
                                      isOutput=True)

    with TileContext(nc) as tc:
        with (
            tc.tile_pool(name="consts", bufs=1) as cpool,
            tc.tile_pool(name="psum", bufs=8, space="PSUM") as ppool,
            tc.tile_pool(name="ostage", bufs=6) as opool,
        ):
            E = cpool.tile([128, H_E * 128], F16)
            E64 = cpool.tile([128, H_E64 * 128], F16)
            Lw = cpool.tile([128, nblk * 128], F16)
            warm = cpool.tile([128, 256], F16)
            wps = ppool.tile([128, 512], F32, tag="ps")

            # PE warm-up during the input DMAs: harmless matmuls on a zeroed
            # tile keep the HAM busy window alive so real matmuls start warm.
            nc.any.memset(warm[:], 0.0)
            for w in range(N_WARM):
                nc.tensor.matmul(wps[:, 0:256], warm[:, :128], warm[:],
                                 start=True, stop=True)

            # E in 2 asymmetric chunks: the first covers the h-blocks the
            # first band's early groups touch, so real matmuls start sooner
            e_flat = ext_in[:].rearrange("p h b -> p (h b)")
            chunk = 15 * 128
            nc.sync.dma_start(out=E[:, 0:chunk], in_=e_flat[:, 0:chunk])
            nc.sync.dma_start(out=E[:, chunk:], in_=e_flat[:, chunk:])
            # E64[p, h] = ext rows (64 + 128h + p), built on device from E.
            # The sem-wait of these triggers head-of-line blocks the HWDGE
            # ring, which (deliberately) gives E exclusive DMA bandwidth.
            e3 = E[:].rearrange("p (h b) -> p h b", b=B)
            e643 = E64[:].rearrange("p (h b) -> p h b", b=B)
            nc.sync.dma_start(out=e643[0:64, :, :], in_=e3[64:128, 0:H_E64, :])
            nc.sync.dma_start(out=e643[64:128, :, :], in_=e3[0:64, 1 : H_E64 + 1, :])

            # all constants stream on the Sync ring in band order, QUEUED
            # BEHIND the E64 triggers whose sem-wait on E head-of-line blocks
            # the ring: E gets exclusive DMA bandwidth, then constants stream
            # at the ring's natural ~0.6us/trigger pace.
            for idx, k in enumerate(order):
                o, q_cnt = offsets[k], plan[k][1]
                # the first two bands' constants ride the (empty) ACT HWDGE
                # ring so they are not FIFO-queued behind E on the SP ring
                eng = nc.scalar if idx < 2 else nc.sync
                eng.dma_start(
                    out=Lw[:, o * 128 : (o + q_cnt) * 128].rearrange(
                        "kk (i r) -> kk i r", r=128
                    ),
                    in_=lhs_in[:, o : o + q_cnt, :],
                )

            for k in order:
                t, q_cnt, s, use64, h_base = plan[k]
                o = offsets[k]
                src = E64 if use64 else E
                h_max = H_E64 if use64 else H_E
                ob = opool.tile([128, GROUPS * 512], F16)
                for g in range(GROUPS):
                    ps = ppool.tile([128, 512], F32)
                    for qi in range(q_cnt):
                        h0 = 4 * g + h_base + qi
                        assert 0 <= h0 and h0 + 4 <= h_max, (k, g, qi, h0)
                        nc.tensor.matmul(
                            ps[:],
                            Lw[:, (o + qi) * 128 : (o + qi + 1) * 128],
                            src[:, h0 * 128 : h0 * 128 + 512],
                            start=(qi == 0),
                            stop=(qi == q_cnt - 1),
                        )
                    # split the PSUM drain across DVE and ACT so neither
                    # engine gates the PSUM bank turnaround
                    base = g * 512
                    nc.vector.tensor_copy(ob[:, base : base + 384], ps[:, 0:384])
                    nc.scalar.copy(ob[:, base + 384 : base + 512], ps[:, 384:512])
                    if k == order[-1]:
                        nc.sync.dma_start(
                            out=out_t[k, :, base : base + 512],
                            in_=ob[:, base : base + 512],
                        )
                # one contiguous 1 MB out-DMA per band (trigger cost ~0.6us
                # on the issuing engine, so fewer+bigger is better); the final
                # band streams per-group so the kernel tail is short
                if k == order[-1]:
                    pass  # handled per-group above
                else:
                    nc.sync.dma_start(out=out_t[k], in_=ob[:])

    nc.compile()
    _program_cache[plan_key] = nc
    return nc


def _maybe_register_trace_hook():
    """Best-effort registration of the axon NTFF profile hook (profiling only;
    harmless no-op if unavailable)."""
    try:
        import sys
        import types

        import antenv

        if getattr(antenv, "axon_hooks", None) is not None:
            return
        from trn_agent_boot.trn_boot import _ntff_profile_via_ctypes

        hooks = types.ModuleType("antenv.axon_hooks")
        hook = _ntff_profile_via_ctypes("/opt/axon/libaxon_pjrt.so")
        hooks.get_axon_ntff_profile_hook = lambda: hook
        hooks.set_axon_ntff_profile_hook = lambda h: None
        antenv.axon_hooks = hooks
        sys.modules["antenv.axon_hooks"] = hooks
    except Exception:
        pass


def kernel(x: np.ndarray, kernels: np.ndarray, padlen) -> np.ndarray:
    global LAST_RESULT
    x = np.asarray(x, dtype=np.float32)
    kernels = np.asarray(kernels, dtype=np.float32)
    assert x.shape == (B, 1, L) and kernels.shape[0] == NB
    assert int(padlen) == P

    plan = _band_plan(kernels)
    plan_key = tuple(plan)
    nc = _build_program(plan_key)

    lhs = np.ascontiguousarray(_toeplitz_blocks(kernels, plan))

    # odd extension + transpose to position-major (ext^T), fp16
    x2d = x[:, 0, :]
    left = 2.0 * x2d[:, :1] - x2d[:, 1 : P + 1][:, ::-1]
    right = 2.0 * x2d[:, -1:] - x2d[:, -P - 1 : -1][:, ::-1]
    ext_t = np.concatenate([left, x2d, right], axis=1).T.astype(np.float16)

    in_maps = []
    for c in range(N_CORES):
        sl = ext_t[c * LC : c * LC + EXT_ROWS]  # (3072, B)
        # SBUF-native layout [p, h, b]: row (128h + p) -> [p, h]
        slp = np.ascontiguousarray(
            sl.reshape(H_E, 128, B).transpose(1, 0, 2)
        )
        in_maps.append({"ext": slp, "lhs": lhs})

    trace = bool(os.environ.get("KERNEL_TRACE"))
    if trace:
        _maybe_register_trace_hook()
    res = run_bass_kernel_spmd(nc, in_maps, list(range(N_CORES)), trace=trace)
    LAST_RESULT = res

    out = np.empty((B, 1, NB, L), np.float32)
    for c in range(N_CORES):
        dev = res.results[c]["out"].astype(np.float32).reshape(NB, 128, GROUPS, 4, 128)
        # dev[k, r, g, j, b] -> out[b, 0, k, c*LC + 512g + 128j + r]
        arr = dev.transpose(4, 0, 2, 3, 1).reshape(B, NB, LC)
        out[:, 0, :, c * LC : (c + 1) * LC] = arr
    return out
